# revision 1
# baseline (speedup 1.0000x reference)
"""CenterPixelCrossAttention Trainium2 kernel.

Math: each batch item has a SINGLE query token (the center pixel), so the
attention collapses to rank-1 form:
    scores[b, t, h] = x[b, t, :] . ck[b, :, h]   with ck[b] = (Wk_h^T q_{b,h}) * sm_scale
    out_v[b, h, :]  = (sum_t softmax_t(scores) * x[b, t, :]) @ Wv_h^T
    out[b]          = concat_h(out_v) @ Wo^T + bo
The full K/V projections (64x the FLOPs) are never materialized and x is
streamed from HBM exactly once (in fp16: half the fp32 traffic; measured
output rel err ~1.5e-4 thanks to fp32 PSUM accumulation everywhere).

Distribution: data-parallel over batch, 2 batch items per NeuronCore, no
collectives.

Device pipeline, per 512-token "quad" (one 768 KB DMA: native [128,2048]
fp16 tokens-on-partitions + d-chunks 2,3 pre-transposed on the host):
  stage 1: PE transposes d-chunks 0,1 (128x128 fp16 is_transpose) -> PSUM,
           DVE copies them to SBUF
  stage 2: scores matmul (ck stationary [128d,8h], x^T moving, N=512),
           fp32 PSUM accumulate over d-chunks
  stage 3: exp on ACT (max-free softmax: logits are O(1) by construction;
           accum_out gives running row sums), PE-transpose of the exp rows
           to attn^T [t, h], one ACT copy out of PSUM
  stage 4: pass-2 matmul (attn^T stationary [128t,8h], native x moving,
           N=512) accumulating unnormalized xbar[h, d] in PSUM
The four stages are software-pipelined across quads (stage s of quad k runs
beside stage s-1 of quad k+1 ...) so the strict-FIFO engines never
head-of-line block on cross-engine handoffs.  Per batch: 1/sum
normalization folds into xbar; Wv (head-block-diagonal) and Wo projections
+ bias run once at the tail.
"""

import numpy as np
from contextlib import ExitStack

import concourse.bass as bass
import concourse.bacc as bacc
import concourse.tile as tile
from concourse import mybir
from concourse.bass_utils import run_bass_kernel_spmd

F32 = mybir.dt.float32
F16 = mybir.dt.float16

B, N, DIM, HEADS, DHEAD = 16, 4096, 512, 8, 64
NCORES = 8
BPC = B // NCORES          # batch items per core = 2
NQ = 8                     # quads (512-token groups) per batch item
QT = 512                   # tokens per quad
NT = 4                     # 128-token sub-tiles per quad
NJ = 4                     # 128-wide d-chunks
NPT = 2                    # d-chunks transposed on the PE (rest on the host)
XW = QT * (NT + NJ - NPT)  # x row width: native 2048 + pre-transposed 1024

TRACE = False              # test.py flips this for profiling runs
LAST_RESULTS = None        # stash of BassKernelResults for test.py


def build_program(reps=1):
    DT = F16
    nc = bacc.Bacc("TRN2", target_bir_lowering=False, debug=False,
                   num_devices=NCORES)

    x_d = nc.dram_tensor("x", [BPC, NQ, 128, XW], DT, kind="ExternalInput")
    ck_d = nc.dram_tensor("ck", [128, BPC * NJ * HEADS], DT, kind="ExternalInput")
    wvt_d = nc.dram_tensor("wvt", [NJ, 128, DIM], DT, kind="ExternalInput")
    wot_d = nc.dram_tensor("wot", [NJ, 128, DIM], DT, kind="ExternalInput")
    bo_d = nc.dram_tensor("bo", [128, NJ], F32, kind="ExternalInput")
    id_d = nc.dram_tensor("ident", [128, 128], DT, kind="ExternalInput")
    out_d = nc.dram_tensor("out", [128, NJ * BPC], F32, kind="ExternalOutput")

    with tile.TileContext(nc) as tc, ExitStack() as ctx:
        const = ctx.enter_context(tc.tile_pool(name="const", bufs=1))
        xq_pool = ctx.enter_context(tc.tile_pool(name="xq", bufs=10))
        xt_pool = ctx.enter_context(tc.tile_pool(name="xt", bufs=8))
        e_pool = ctx.enter_context(tc.tile_pool(name="e", bufs=4))
        at_pool = ctx.enter_context(tc.tile_pool(name="at", bufs=6))
        ps_xt = ctx.enter_context(tc.tile_pool(name="ps_xt", bufs=3, space="PSUM"))
        ps_sc = ctx.enter_context(tc.tile_pool(name="ps_sc", bufs=3, space="PSUM"))
        ps_at = ps_xt
        ps_xb = ctx.enter_context(tc.tile_pool(name="ps_xb", bufs=2, space="PSUM"))

        ident = const.tile([128, 128], DT)
        nc.scalar.dma_start(ident[:], id_d.ap()[:, :])
        ck_sb = const.tile([128, BPC * NJ * HEADS], DT)
        nc.scalar.dma_start(ck_sb[:], ck_d.ap()[:, :])
        wvt_sb = const.tile([128, NJ * DIM], DT)
        wot_sb = const.tile([128, NJ * DIM], DT)
        bo_sb = const.tile([128, NJ], F32)

        sums_sb = const.tile([8, BPC * NQ], F32)
        sinv_sb = const.tile([8, BPC], F32)
        junk_sb = const.tile([8, NQ], F32)
        xbar_sb = const.tile([8, BPC * DIM], DT)
        xbarT = const.tile([128, NJ * BPC * HEADS], DT)
        v_all = const.tile([128, NJ * BPC], DT)
        o_sb = const.tile([128, NJ * BPC], F32)

        for _rep in range(reps):
            ps_xbar = [ps_xb.tile([8, DIM], F32, tag="xbar", name=f"xbar{i}")
                       for i in range(BPC)]

            def scores_phase(b, q, ps_s, xts):
                # scores accumulate over d-chunks: [8, 512]
                for j in range(NJ):
                    nc.tensor.matmul(
                        ps_s[:],
                        ck_sb[:, (b * NJ + j) * HEADS:(b * NJ + j + 1) * HEADS],
                        xts[j][:],
                        start=(j == 0),
                        stop=(j == NJ - 1),
                    )

            def attn_a(b, q, ps_s):
                # exp (+ running row-sums), transpose attn to [t, h]
                e_sb = e_pool.tile([8, QT], DT, tag="e", name="e_sb")
                nc.scalar.activation(
                    e_sb[:], ps_s[:], mybir.ActivationFunctionType.Exp,
                    accum_out=sums_sb[:, b * NQ + q: b * NQ + q + 1],
                )
                pat = ps_at.tile([128, NT * 8], DT, tag="pxt", name="pat")
                for s in range(NT):
                    nc.tensor.matmul(
                        pat[:, s * 8:(s + 1) * 8],
                        e_sb[:, s * 128:(s + 1) * 128],
                        ident[0:8, 0:8],
                        is_transpose=True,
                    )
                at_sb = at_pool.tile([128, NT * 8], DT, tag="at", name="at_sb")
                nc.vector.tensor_copy(at_sb[:], pat[:])
                return at_sb

            def attn_b(b, q, at_sb, xq):
                for s in range(NT):
                    # xbar[h, d] += attn^T.T @ x  (contraction over tokens)
                    nc.tensor.matmul(
                        ps_xbar[b][:],
                        at_sb[:, s * 8:(s + 1) * 8],
                        xq[:, s * QT:(s + 1) * QT],
                        start=(q == 0 and s == 0),
                        stop=(q == NQ - 1 and s == NT - 1),
                    )

            def batch_tail(b):
                # sums -> 1/sum, normalize xbar, transpose to [d, (b,h)]
                nc.vector.tensor_scalar(
                    junk_sb[:], sums_sb[:, b * NQ:(b + 1) * NQ], 0.0, None,
                    mybir.AluOpType.add, mybir.AluOpType.add,
                    accum_out=sinv_sb[:, b:b + 1],
                )
                nc.vector.reciprocal(sinv_sb[:, b:b + 1], sinv_sb[:, b:b + 1])
                nc.vector.tensor_scalar_mul(
                    xbar_sb[:, b * DIM:(b + 1) * DIM], ps_xbar[b][:],
                    sinv_sb[:, b:b + 1],
                )
                for j in range(NJ):
                    pt = ps_at.tile([128, NT * 8], DT, tag="pxt", name="pt")
                    nc.tensor.matmul(
                        pt[:, 0:8],
                        xbar_sb[:, b * DIM + j * 128: b * DIM + (j + 1) * 128],
                        ident[0:8, 0:8],
                        is_transpose=True,
                    )
                    nc.scalar.copy(
                        xbarT[:, j * BPC * 8 + b * 8: j * BPC * 8 + (b + 1) * 8],
                        pt[:, 0:8])

            # 4-stage software pipeline over quads (see module docstring)
            S1 = S2 = S3 = None
            quads = [(b, q) for b in range(BPC) for q in range(NQ)]
            for item in quads + [None] * 3:
                if item is not None:
                    b, q = item
                    xq = xq_pool.tile([128, XW], DT, tag="xq")
                    if q == 0 and b == 0:
                        # split the very first load so the PE starts sooner
                        W = XW // 4
                        for s in range(4):
                            nc.sync.dma_start(
                                xq[:, s * W:(s + 1) * W],
                                x_d.ap()[b, q][:, s * W:(s + 1) * W])
                    else:
                        nc.sync.dma_start(xq[:], x_d.ap()[b, q])

                    ps_s = ps_sc.tile([8, QT], F32, tag="sc", name="ps_s")
                    xts = []
                    for j in range(NJ):
                        if j >= NPT:
                            # chunks >= NPT sit pre-transposed in the load tail
                            k = NT + (j - NPT)
                            xts.append(xq[:, k * QT:(k + 1) * QT])
                            continue
                        xt = xt_pool.tile([128, QT], DT, tag="xt", name="xt")
                        pxt = ps_xt.tile([128, QT], DT, tag="pxt", name="pxt")
                        for s in range(NT):
                            # psum slot s = (x chunk [t,d]).T
                            nc.tensor.matmul(
                                pxt[:, s * 128:(s + 1) * 128],
                                xq[:, s * QT + j * 128: s * QT + (j + 1) * 128],
                                ident[:],
                                is_transpose=True,
                            )
                        nc.vector.tensor_copy(xt[:], pxt[:])
                        xts.append(xt)
                    new_S1 = (b, q, ps_s, xts, xq)
                else:
                    new_S1 = None

                if S1 is not None:
                    scores_phase(*S1[:4])
                new_S3 = None
                if S2 is not None:
                    at_sb = attn_a(S2[0], S2[1], S2[2])
                    new_S3 = (S2[0], S2[1], at_sb, S2[3])
                if S3 is not None:
                    attn_b(*S3)
                    if S3[1] == NQ - 1:
                        batch_tail(S3[0])
                S3 = new_S3
                S2 = (S1[0], S1[1], S1[2], S1[4]) if S1 is not None else None
                S1 = new_S1

            for j in range(NJ):
                nc.scalar.dma_start(wvt_sb[:, j * DIM:(j + 1) * DIM], wvt_d.ap()[j])
                nc.scalar.dma_start(wot_sb[:, j * DIM:(j + 1) * DIM], wot_d.ap()[j])
            nc.scalar.dma_start(bo_sb[:], bo_d.ap()[:, :])

            # v projection: v[i, (b,h)] = sum_d WvT[d, i] * xbarT[d, (b,h)]
            for ji in range(NJ):
                pv = ps_xt.tile([128, BPC * HEADS], F32, tag="pxt", name="pv")
                for jd in range(NJ):
                    nc.tensor.matmul(
                        pv[:],
                        wvt_sb[:, jd * DIM + ji * 128: jd * DIM + (ji + 1) * 128],
                        xbarT[:, jd * BPC * 8:(jd + 1) * BPC * 8],
                        start=(jd == 0),
                        stop=(jd == NJ - 1),
                    )
                # head-diagonal extraction: i-chunk ji covers heads 2ji
                # (rows 0-63) and 2ji+1 (rows 64-127); batch b at column b*8+h
                for bb in range(BPC):
                    nc.vector.tensor_copy(
                        v_all[0:64, ji * BPC + bb: ji * BPC + bb + 1],
                        pv[0:64, bb * 8 + 2 * ji: bb * 8 + 2 * ji + 1],
                    )
                    nc.vector.tensor_copy(
                        v_all[64:128, ji * BPC + bb: ji * BPC + bb + 1],
                        pv[64:128, bb * 8 + 2 * ji + 1: bb * 8 + 2 * ji + 2],
                    )

            # out projection: out[dim, b] = sum_i WoT[i, dim] * v[i, b]
            for jd in range(NJ):
                po = ps_sc.tile([128, BPC], F32, tag="sc", name="po")
                for ji in range(NJ):
                    nc.tensor.matmul(
                        po[:],
                        wot_sb[:, ji * DIM + jd * 128: ji * DIM + (jd + 1) * 128],
                        v_all[:, ji * BPC:(ji + 1) * BPC],
                        start=(ji == 0),
                        stop=(ji == NJ - 1),
                    )
                nc.scalar.activation(
                    o_sb[:, jd * BPC:(jd + 1) * BPC], po[:],
                    mybir.ActivationFunctionType.Identity,
                    bias=bo_sb[:, jd:jd + 1],
                )
            nc.sync.dma_start(out_d.ap()[:, :], o_sb[:])

    nc.compile()
    return nc


def kernel(**inputs):
    global LAST_RESULTS
    x = np.ascontiguousarray(np.asarray(inputs["x"], dtype=np.float32))
    Wq = np.asarray(inputs["Wq"], dtype=np.float32)
    Wk = np.asarray(inputs["Wk"], dtype=np.float32)
    Wv = np.asarray(inputs["Wv"], dtype=np.float32)
    Wo = np.asarray(inputs["Wo"], dtype=np.float32)
    bo = np.asarray(inputs["bo"], dtype=np.float32)
    pi = np.asarray(inputs["patch_indices"]).astype(np.int64)
    scale = np.asarray(inputs["scale"]).astype(np.int64)

    idx = pi[:, 0] * scale[1] + pi[:, 1]
    sel = x[np.arange(B), idx]                       # [B, DIM]
    q = (sel @ Wq.T).reshape(B, HEADS, DHEAD)        # [B, h, dh]
    # ck[b, d, h] = sum_i q[b,h,i] * Wk[h*64+i, d], scaled by 1/sqrt(dh)
    ck = np.einsum("bhi,hid->bdh", q, Wk.reshape(HEADS, DHEAD, DIM),
                   dtype=np.float32).astype(np.float32) * np.float32(DHEAD ** -0.5)

    wvt = np.ascontiguousarray(Wv.T.reshape(NJ, 128, DIM)).astype(np.float16)
    wot = np.ascontiguousarray(Wo.T.reshape(NJ, 128, DIM)).astype(np.float16)
    bo_r = np.ascontiguousarray(bo.reshape(NJ, 128).T)
    ident = np.eye(128, dtype=np.float16)

    x16 = x.astype(np.float16)
    in_maps = []
    for c in range(NCORES):
        xsf = x16[c * BPC:(c + 1) * BPC]             # [2, 4096, 512] fp16
        xs_nat = xsf.reshape(BPC, NQ, NT, 128, DIM).transpose(0, 1, 3, 2, 4)
        xs_nat = xs_nat.reshape(BPC, NQ, 128, NT * DIM)
        # d-chunks >= NPT pre-transposed to [d, t] on the host
        xtt = xsf.reshape(BPC, NQ, QT, DIM)[:, :, :, NPT * 128:]
        xtt = xtt.transpose(0, 1, 3, 2)              # [2, 8, (NJ-NPT)*128, 512]
        xtt = xtt.reshape(BPC, NQ, NJ - NPT, 128, QT).transpose(0, 1, 3, 2, 4)
        xtt = xtt.reshape(BPC, NQ, 128, (NJ - NPT) * QT)
        xs = np.ascontiguousarray(np.concatenate([xs_nat, xtt], axis=3))
        ckc = np.empty((128, BPC * NJ * HEADS), dtype=np.float16)
        for bb in range(BPC):
            for j in range(NJ):
                ckc[:, (bb * NJ + j) * HEADS:(bb * NJ + j + 1) * HEADS] = \
                    ck[c * BPC + bb, j * 128:(j + 1) * 128, :]
        in_maps.append({
            "x": xs, "ck": ckc, "wvt": wvt, "wot": wot,
            "bo": bo_r, "ident": ident,
        })

    nc = build_program()
    res = run_bass_kernel_spmd(nc, in_maps, list(range(NCORES)), trace=TRACE)
    LAST_RESULTS = res

    out = np.empty((B, 1, DIM), dtype=np.float32)
    for c in range(NCORES):
        oc = res.results[c]["out"]                   # [128, NJ*BPC]
        for bb in range(BPC):
            out[c * BPC + bb, 0, :] = oc[:, bb::BPC].T.reshape(DIM)
    return out



# revision 4
# speedup vs baseline: 1.3228x; 1.3228x over previous
"""CenterPixelCrossAttention Trainium2 kernel (v3).

Math: each batch item has a SINGLE query token (the center pixel), so the
attention collapses to rank-1 form:
    scores[b, t, h] = x[b, t, :] . ck[b, :, h]   with ck[b] = (Wk_h^T q_{b,h}) * sm_scale
    xbar[b, h, :]   = sum_t exp(scores[b,t,h]) * x[b, t, :]   (unnormalized)
    out[b]          = concat_h((Wv_h @ xbar_{b,h}) / S_{b,h}) @ Wo^T + bo
The full K/V projections are never materialized; x streams from HBM exactly
once in fp16 NATIVE layout only (8.4 MB/core, the model-bandwidth floor at
~360 GB/s).

Key structure:
  - every x-consuming matmul has out free size 8 (stationary loads are
    cheap; cost ~ output columns):
      scores^T[t, h]: lhsT = x^T chunk [128d, 128t], rhs = ck [128d, 8h]
      xbar^T [d, h]:  lhsT = x  chunk [128t, 128d], rhs = at [128t, 8h]
      sums  [1, h]:   lhsT = ones [128t, 1],        rhs = at [128t, 8h]
  - x^T chunks are produced on-chip: PE is_transpose into PSUM, DVE copies
    to SBUF; ACT only does the exp.  All weights/constants arrive in two
    blob DMAs so the sequencer issues the 16 x-quad streams back to back.
  - xbar/sums PSUM accumulators live across a whole batch; only the very
    first matmul touching the bank uses start=True and only the very last
    uses stop=True (PSUM pending-zero is per byte, so each column group
    zero-fills lazily on first touch and accumulates afterwards).
  - softmax is max-free (logits are O(1) by construction); 1/sum folds in
    at the per-batch tail via a ones-broadcast matmul + DVE multiply.

Distribution: data-parallel over batch, 2 batch items per NeuronCore, no
collectives.  3-stage software pipeline over 16 (batch, quad) items:
  A(k): DMA quad k            B(k): PE transposes + DVE copies
  C(k): scores + exp          D(k): xbar/sums accumulation (+ batch tail)
"""

import numpy as np
from contextlib import ExitStack

import concourse.bass as bass
import concourse.bacc as bacc
import concourse.tile as tile
from concourse import mybir
from concourse.bass_utils import run_bass_kernel_spmd

F32 = mybir.dt.float32
F16 = mybir.dt.float16

B, N, DIM, HEADS, DHEAD = 16, 4096, 512, 8, 64
NCORES = 8
BPC = B // NCORES          # batch items per core = 2
NQ = 8                     # quads (512-token groups) per batch item
QT = 512                   # tokens per quad
NT = 4                     # 128-token sub-tiles per quad
NJ = 4                     # 128-wide d-chunks
QW = QT * NT               # 2048 fp16 columns per quad (native layout only)

# f16 constant blob column offsets: ident | ck | ones | wvt | wot
C_ID = 0
C_CK = 128
C_ONES = C_CK + BPC * NJ * HEADS      # 192
C_WVT = C_ONES + 1                    # 193
C_WOT = C_WVT + NJ * DIM              # 2241
W16 = C_WOT + NJ * DIM                # 4289
# f32 constant blob: bo (4 cols) | onesbc row (128 cols, row 0 only)
C_BO = 0
C_OBC = NJ                            # 4
W32 = NJ + 128                        # 132

TRACE = False              # test.py flips this for profiling runs
LAST_RESULTS = None        # stash of BassKernelResults for test.py


def build_program(reps=1):
    DT = F16
    nc = bacc.Bacc("TRN2", target_bir_lowering=False, debug=False,
                   num_devices=NCORES)

    x_d = nc.dram_tensor("x", [BPC, NQ, 128, QW], DT, kind="ExternalInput")
    c16_d = nc.dram_tensor("c16", [128, W16], DT, kind="ExternalInput")
    c32_d = nc.dram_tensor("c32", [128, W32], F32, kind="ExternalInput")
    out_d = nc.dram_tensor("out", [128, NJ * BPC], F32, kind="ExternalOutput")

    with tile.TileContext(nc) as tc, ExitStack() as ctx:
        const = ctx.enter_context(tc.tile_pool(name="const", bufs=1))
        xq_pool = ctx.enter_context(tc.tile_pool(name="xq", bufs=BPC * NQ))
        xt_pool = ctx.enter_context(tc.tile_pool(name="xt", bufs=BPC * NQ))
        at_pool = ctx.enter_context(tc.tile_pool(name="at", bufs=4))
        ps_big = ctx.enter_context(tc.tile_pool(name="ps_big", bufs=4, space="PSUM"))
        ps_sm = ctx.enter_context(tc.tile_pool(name="ps_sm", bufs=2, space="PSUM"))
        ps_acc = ctx.enter_context(tc.tile_pool(name="ps_acc", bufs=2, space="PSUM"))

        c16 = const.tile([128, W16], DT)
        c32 = const.tile([128, W32], F32)
        ident = c16[:, C_ID:C_ID + 128]
        ones_sb = c16[:, C_ONES:C_ONES + 1]
        obc_sb = c32[0:1, C_OBC:C_OBC + 128]

        sums_sb = const.tile([1, BPC * HEADS], F32)
        sbc_sb = const.tile([128, BPC * HEADS], F32)
        xbarT = const.tile([128, BPC * NJ * HEADS], DT)
        v_all = const.tile([128, BPC * NJ], DT)
        o_sb = const.tile([128, BPC * NJ], F32)

        # two blob DMAs for every weight/constant: keeps the SP sequencer
        # free to issue the 16 x-quad streams back to back afterwards
        nc.sync.dma_start(c16[:], c16_d.ap()[:, :])
        nc.sync.dma_start(c32[:], c32_d.ap()[:, :])

        for _rep in range(reps):
            xqs = {}
            xts = {}
            ats = {}
            accs = {}

            def stage_a(k):
                b, q = divmod(k, NQ)
                xq = xq_pool.tile([128, QW], DT, tag="xq")
                xqs[k] = xq
                nc.sync.dma_start(xq[:], x_d.ap()[b, q])

            def stage_b(k):
                xq = xqs[k]
                xt = xt_pool.tile([128, QW], DT, tag="xt")
                xts[k] = xt
                for jh in range(2):          # two half-quads: j in {0,1}, {2,3}
                    pb = ps_big.tile([128, QW // 2], DT, tag="pb", name="pb")
                    for jj in range(2):
                        j = jh * 2 + jj
                        for s in range(NT):
                            nc.tensor.matmul(
                                pb[:, jj * QT + s * 128: jj * QT + (s + 1) * 128],
                                xq[:, s * DIM + j * 128: s * DIM + (j + 1) * 128],
                                ident,
                                is_transpose=True,
                            )
                    nc.vector.tensor_copy(
                        xt[:, jh * QT * 2:(jh + 1) * QT * 2], pb[:])

            def stage_c(k):
                b, q = divmod(k, NQ)
                xt = xts[k]
                ps_s = ps_sm.tile([128, 32], F32, tag="sm", name="ps_s")
                for s in range(NT):
                    for j in range(NJ):
                        nc.tensor.matmul(
                            ps_s[:, s * 8:(s + 1) * 8],
                            xt[:, j * QT + s * 128: j * QT + (s + 1) * 128],
                            c16[:, C_CK + (b * NJ + j) * 8: C_CK + (b * NJ + j + 1) * 8],
                            start=(j == 0),
                            stop=(j == NJ - 1),
                        )
                at = at_pool.tile([128, 32], DT, tag="at", name="at")
                ats[k] = at
                nc.scalar.activation(at[:], ps_s[:],
                                     mybir.ActivationFunctionType.Exp)

            def stage_d(k):
                b, q = divmod(k, NQ)
                xq, at = xqs[k], ats[k]
                if q == 0:
                    accs[b] = ps_acc.tile([128, 64], F32, tag="acc",
                                          name=f"acc{b}")
                acc = accs[b]
                # one PSUM bank, five column groups (4x xbar_j + sums).
                # start=True only on the very first matmul touching the bank
                # (marks the whole zero region pending; each group then
                # lazily zero-fills its own bytes on first touch), stop=True
                # only on the very last.
                for s in range(NT):
                    last = (q == NQ - 1 and s == NT - 1)
                    if not last:
                        # sums after the j-loop in steady state
                        for j in range(NJ):
                            nc.tensor.matmul(
                                acc[:, j * 8:(j + 1) * 8],
                                xq[:, s * DIM + j * 128: s * DIM + (j + 1) * 128],
                                at[:, s * 8:(s + 1) * 8],
                                start=(q == 0 and s == 0 and j == 0),
                                stop=False,
                            )
                        nc.tensor.matmul(acc[0:1, 32:40], ones_sb,
                                         at[:, s * 8:(s + 1) * 8],
                                         start=False, stop=False)
                    else:
                        # final step: sums first, then close the group with a
                        # full-128-partition matmul so every partition's zero
                        # region is released before the tail reads it
                        nc.tensor.matmul(acc[0:1, 32:40], ones_sb,
                                         at[:, s * 8:(s + 1) * 8],
                                         start=False, stop=False)
                        for j in range(NJ):
                            nc.tensor.matmul(
                                acc[:, j * 8:(j + 1) * 8],
                                xq[:, s * DIM + j * 128: s * DIM + (j + 1) * 128],
                                at[:, s * 8:(s + 1) * 8],
                                start=False,
                                stop=(j == NJ - 1),
                            )

            def batch_tail(b):
                acc = accs[b]
                h0 = b * HEADS
                nc.vector.tensor_copy(sums_sb[0:1, h0:h0 + 8], acc[0:1, 32:40])
                ps_bc = ps_sm.tile([128, 32], F32, tag="sm", name="ps_bc")
                nc.tensor.matmul(ps_bc[:, 0:8], obc_sb,
                                 sums_sb[0:1, h0:h0 + 8])
                nc.vector.reciprocal(sbc_sb[:, h0:h0 + 8], ps_bc[:, 0:8])
                for j in range(NJ):
                    nc.vector.tensor_mul(
                        xbarT[:, b * 32 + j * 8: b * 32 + (j + 1) * 8],
                        acc[:, j * 8:(j + 1) * 8],
                        sbc_sb[:, h0:h0 + 8],
                    )
                for ji in range(NJ):
                    pv = ps_sm.tile([128, 32], F32, tag="sm", name="pv")
                    for jd in range(NJ):
                        nc.tensor.matmul(
                            pv[:, 0:8],
                            c16[:, C_WVT + jd * DIM + ji * 128: C_WVT + jd * DIM + (ji + 1) * 128],
                            xbarT[:, b * 32 + jd * 8: b * 32 + (jd + 1) * 8],
                            start=(jd == 0),
                            stop=(jd == NJ - 1),
                        )
                    # head-block-diagonal extract: chunk ji covers heads 2ji
                    # (rows 0-63) and 2ji+1 (rows 64-127)
                    c = b * NJ + ji
                    nc.vector.tensor_copy(v_all[0:64, c:c + 1],
                                          pv[0:64, 2 * ji:2 * ji + 1])
                    nc.vector.tensor_copy(v_all[64:128, c:c + 1],
                                          pv[64:128, 2 * ji + 1:2 * ji + 2])
                for jd in range(NJ):
                    po = ps_sm.tile([128, 32], F32, tag="sm", name="po")
                    for ji in range(NJ):
                        nc.tensor.matmul(
                            po[:, 0:1],
                            c16[:, C_WOT + ji * DIM + jd * 128: C_WOT + ji * DIM + (jd + 1) * 128],
                            v_all[:, b * NJ + ji: b * NJ + ji + 1],
                            start=(ji == 0),
                            stop=(ji == NJ - 1),
                        )
                    nc.vector.tensor_tensor(
                        o_sb[:, b * NJ + jd: b * NJ + jd + 1],
                        po[:, 0:1], c32[:, C_BO + jd:C_BO + jd + 1],
                        mybir.AluOpType.add,
                    )

            nitems = BPC * NQ
            for i in range(nitems + 3):
                if i < nitems:
                    stage_a(i)
                if 1 <= i < nitems + 1:
                    stage_b(i - 1)
                if 2 <= i < nitems + 2:
                    stage_c(i - 2)
                if 3 <= i < nitems + 3:
                    k = i - 3
                    stage_d(k)
                    b, q = divmod(k, NQ)
                    if q == NQ - 1:
                        batch_tail(b)
                # batch-0 output DMA a few iterations after its tail so the
                # SP sequencer never parks on it
                if i == nitems - 2:
                    nc.sync.dma_start(out_d.ap()[:, 0:NJ], o_sb[:, 0:NJ])
            nc.sync.dma_start(out_d.ap()[:, NJ:2 * NJ], o_sb[:, NJ:2 * NJ])

    nc.compile()
    return nc


def kernel(**inputs):
    global LAST_RESULTS
    x = np.ascontiguousarray(np.asarray(inputs["x"], dtype=np.float32))
    Wq = np.asarray(inputs["Wq"], dtype=np.float32)
    Wk = np.asarray(inputs["Wk"], dtype=np.float32)
    Wv = np.asarray(inputs["Wv"], dtype=np.float32)
    Wo = np.asarray(inputs["Wo"], dtype=np.float32)
    bo = np.asarray(inputs["bo"], dtype=np.float32)
    pi = np.asarray(inputs["patch_indices"]).astype(np.int64)
    scale = np.asarray(inputs["scale"]).astype(np.int64)

    idx = pi[:, 0] * scale[1] + pi[:, 1]
    sel = x[np.arange(B), idx]                       # [B, DIM]
    q = (sel @ Wq.T).reshape(B, HEADS, DHEAD)        # [B, h, dh]
    # ck[b, d, h] = sum_i q[b,h,i] * Wk[h*64+i, d], scaled by 1/sqrt(dh)
    ck = np.einsum("bhi,hid->bdh", q, Wk.reshape(HEADS, DHEAD, DIM),
                   dtype=np.float32).astype(np.float32) * np.float32(DHEAD ** -0.5)

    wvt = Wv.T.reshape(NJ, 128, DIM).transpose(1, 0, 2).reshape(128, NJ * DIM)
    wot = Wo.T.reshape(NJ, 128, DIM).transpose(1, 0, 2).reshape(128, NJ * DIM)

    c32 = np.zeros((128, W32), dtype=np.float32)
    c32[:, C_BO:C_BO + NJ] = bo.reshape(NJ, 128).T
    c32[0, C_OBC:C_OBC + 128] = 1.0

    x16 = x.astype(np.float16)
    in_maps = []
    for c in range(NCORES):
        xsf = x16[c * BPC:(c + 1) * BPC]             # [2, 4096, 512] fp16
        xs_nat = xsf.reshape(BPC, NQ, NT, 128, DIM).transpose(0, 1, 3, 2, 4)
        xs = np.ascontiguousarray(xs_nat.reshape(BPC, NQ, 128, NT * DIM))
        c16 = np.zeros((128, W16), dtype=np.float16)
        c16[:, C_ID:C_ID + 128] = np.eye(128, dtype=np.float16)
        for bb in range(BPC):
            for j in range(NJ):
                c16[:, C_CK + (bb * NJ + j) * HEADS:C_CK + (bb * NJ + j + 1) * HEADS] = \
                    ck[c * BPC + bb, j * 128:(j + 1) * 128, :].astype(np.float16)
        c16[:, C_ONES] = 1.0
        c16[:, C_WVT:C_WVT + NJ * DIM] = wvt.astype(np.float16)
        c16[:, C_WOT:C_WOT + NJ * DIM] = wot.astype(np.float16)
        in_maps.append({"x": xs, "c16": c16, "c32": c32})

    nc = build_program()
    res = run_bass_kernel_spmd(nc, in_maps, list(range(NCORES)), trace=TRACE)
    LAST_RESULTS = res

    out = np.empty((B, 1, DIM), dtype=np.float32)
    for c in range(NCORES):
        oc = res.results[c]["out"]                   # [128, NJ*BPC]
        for bb in range(BPC):
            out[c * BPC + bb, 0, :] = oc[:, bb * NJ:(bb + 1) * NJ].T.reshape(DIM)
    return out


# revision 6
# speedup vs baseline: 1.4023x; 1.0601x over previous
"""CenterPixelCrossAttention Trainium2 kernel (v3).

Math: each batch item has a SINGLE query token (the center pixel), so the
attention collapses to rank-1 form:
    scores[b, t, h] = x[b, t, :] . ck[b, :, h]   with ck[b] = (Wk_h^T q_{b,h}) * sm_scale
    xbar[b, h, :]   = sum_t exp(scores[b,t,h]) * x[b, t, :]   (unnormalized)
    out[b]          = concat_h((Wv_h @ xbar_{b,h}) / S_{b,h}) @ Wo^T + bo
The full K/V projections are never materialized; x streams from HBM exactly
once in fp16 NATIVE layout only (8.4 MB/core, the model-bandwidth floor at
~360 GB/s).

Key structure:
  - every x-consuming matmul has out free size 8 (stationary loads are
    cheap; cost ~ output columns):
      scores^T[t, h]: lhsT = x^T chunk [128d, 128t], rhs = ck [128d, 8h]
      xbar^T [d, h]:  lhsT = x  chunk [128t, 128d], rhs = at [128t, 8h]
      sums  [1, h]:   lhsT = ones [128t, 1],        rhs = at [128t, 8h]
  - x^T chunks are produced on-chip: PE is_transpose into PSUM, DVE copies
    to SBUF; ACT only does the exp.  All weights/constants arrive in two
    blob DMAs so the sequencer issues the 16 x-quad streams back to back.
  - xbar/sums PSUM accumulators live across a whole batch; only the very
    first matmul touching the bank uses start=True and only the very last
    uses stop=True (PSUM pending-zero is per byte, so each column group
    zero-fills lazily on first touch and accumulates afterwards).
  - softmax is max-free (logits are O(1) by construction); 1/sum folds in
    at the per-batch tail via a ones-broadcast matmul + DVE multiply.

Distribution: data-parallel over batch, 2 batch items per NeuronCore, no
collectives.  3-stage software pipeline over 16 (batch, quad) items:
  A(k): DMA quad k            B(k): PE transposes + DVE copies
  C(k): scores + exp          D(k): xbar/sums accumulation (+ batch tail)
"""

import numpy as np
from contextlib import ExitStack

import concourse.bass as bass
import concourse.bacc as bacc
import concourse.tile as tile
from concourse import mybir
from concourse.bass_utils import run_bass_kernel_spmd

F32 = mybir.dt.float32
F16 = mybir.dt.float16

B, N, DIM, HEADS, DHEAD = 16, 4096, 512, 8, 64
NCORES = 8
BPC = B // NCORES          # batch items per core = 2
NQ = 8                     # quads (512-token groups) per batch item
QT = 512                   # tokens per quad
NT = 4                     # 128-token sub-tiles per quad
NJ = 4                     # 128-wide d-chunks
QW = QT * NT               # 2048 fp16 columns per quad (native layout only)

# f16 constant blob column offsets: ident | ck | ones | wvt | wot
C_ID = 0
C_CK = 128
C_ONES = C_CK + BPC * NJ * HEADS      # 192
C_WVT = C_ONES + 1                    # 193
C_WOT = C_WVT + NJ * DIM              # 2241
W16 = C_WOT + NJ * DIM                # 4289
# f32 constant blob: bo (4 cols) | onesbc row (128 cols, row 0 only)
C_BO = 0
C_OBC = NJ                            # 4
W32 = NJ + 128                        # 132

TRACE = False              # test.py flips this for profiling runs
LAST_RESULTS = None        # stash of BassKernelResults for test.py


def build_program(reps=1):
    DT = F16
    nc = bacc.Bacc("TRN2", target_bir_lowering=False, debug=False,
                   num_devices=NCORES)

    x_d = nc.dram_tensor("x", [BPC, NQ, 128, QW], DT, kind="ExternalInput")
    c16_d = nc.dram_tensor("c16", [128, W16], DT, kind="ExternalInput")
    c32_d = nc.dram_tensor("c32", [128, W32], F32, kind="ExternalInput")
    out_d = nc.dram_tensor("out", [128, NJ * BPC], F32, kind="ExternalOutput")

    with tile.TileContext(nc) as tc, ExitStack() as ctx:
        const = ctx.enter_context(tc.tile_pool(name="const", bufs=1))
        xq_pool = ctx.enter_context(tc.tile_pool(name="xq", bufs=BPC * NQ))
        xt_pool = ctx.enter_context(tc.tile_pool(name="xt", bufs=BPC * NQ))
        at_pool = ctx.enter_context(tc.tile_pool(name="at", bufs=4))
        ps_big = ctx.enter_context(tc.tile_pool(name="ps_big", bufs=4, space="PSUM"))
        ps_sm = ctx.enter_context(tc.tile_pool(name="ps_sm", bufs=2, space="PSUM"))
        ps_acc = ctx.enter_context(tc.tile_pool(name="ps_acc", bufs=2, space="PSUM"))

        c16 = const.tile([128, W16], DT)
        c32 = const.tile([128, W32], F32)
        ident = c16[:, C_ID:C_ID + 128]
        ones_sb = c16[:, C_ONES:C_ONES + 1]
        obc_sb = c32[0:1, C_OBC:C_OBC + 128]

        sums_sb = const.tile([1, BPC * HEADS], F32)
        sbc_sb = const.tile([128, BPC * HEADS], F32)
        xbarT = const.tile([128, BPC * NJ * HEADS], DT)
        v_all = const.tile([128, BPC * NJ], DT)
        o_sb = const.tile([128, BPC * NJ], F32)

        # two blob DMAs for every weight/constant: keeps the SP sequencer
        # free to issue the 16 x-quad streams back to back afterwards
        nc.sync.dma_start(c16[:], c16_d.ap()[:, :])
        nc.sync.dma_start(c32[:], c32_d.ap()[:, :])

        for _rep in range(reps):
            xqs = {}
            xts = {}
            ats = {}
            accs = {}

            def stage_a(k):
                b, q = divmod(k, NQ)
                xq = xq_pool.tile([128, QW], DT, tag="xq")
                xqs[k] = xq
                nc.sync.dma_start(xq[:], x_d.ap()[b, q])

            def stage_b(k):
                xq = xqs[k]
                xt = xt_pool.tile([128, QW], DT, tag="xt")
                xts[k] = xt
                for jh in range(2):          # two half-quads: j in {0,1}, {2,3}
                    pb = ps_big.tile([128, QW // 2], DT, tag="pb", name="pb")
                    for jj in range(2):
                        j = jh * 2 + jj
                        for s in range(NT):
                            nc.tensor.matmul(
                                pb[:, jj * QT + s * 128: jj * QT + (s + 1) * 128],
                                xq[:, s * DIM + j * 128: s * DIM + (j + 1) * 128],
                                ident,
                                is_transpose=True,
                            )
                    if jh == 0:
                        nc.vector.tensor_copy(xt[:, 0:QT * 2], pb[:])
                    else:
                        nc.scalar.copy(xt[:, QT * 2:QT * 4], pb[:])

            def stage_c(k):
                b, q = divmod(k, NQ)
                xt = xts[k]
                ps_s = ps_sm.tile([128, 32], F32, tag="sm", name="ps_s")
                for s in range(NT):
                    for j in range(NJ):
                        nc.tensor.matmul(
                            ps_s[:, s * 8:(s + 1) * 8],
                            xt[:, j * QT + s * 128: j * QT + (s + 1) * 128],
                            c16[:, C_CK + (b * NJ + j) * 8: C_CK + (b * NJ + j + 1) * 8],
                            start=(j == 0),
                            stop=(j == NJ - 1),
                        )
                at = at_pool.tile([128, 32], DT, tag="at", name="at")
                ats[k] = at
                nc.scalar.activation(at[:], ps_s[:],
                                     mybir.ActivationFunctionType.Exp)

            def stage_d(k):
                b, q = divmod(k, NQ)
                xq, at = xqs[k], ats[k]
                if q == 0:
                    accs[b] = ps_acc.tile([128, 64], F32, tag="acc",
                                          name=f"acc{b}")
                acc = accs[b]
                # one PSUM bank, five column groups (4x xbar_j + sums).
                # start=True only on the very first matmul touching the bank
                # (marks the whole zero region pending; each group then
                # lazily zero-fills its own bytes on first touch), stop=True
                # only on the very last.
                for s in range(NT):
                    last = (q == NQ - 1 and s == NT - 1)
                    if not last:
                        # sums after the j-loop in steady state
                        for j in range(NJ):
                            nc.tensor.matmul(
                                acc[:, j * 8:(j + 1) * 8],
                                xq[:, s * DIM + j * 128: s * DIM + (j + 1) * 128],
                                at[:, s * 8:(s + 1) * 8],
                                start=(q == 0 and s == 0 and j == 0),
                                stop=False,
                            )
                        nc.tensor.matmul(acc[0:1, 32:40], ones_sb,
                                         at[:, s * 8:(s + 1) * 8],
                                         start=False, stop=False)
                    else:
                        # final step: sums first, then close the group with a
                        # full-128-partition matmul so every partition's zero
                        # region is released before the tail reads it
                        nc.tensor.matmul(acc[0:1, 32:40], ones_sb,
                                         at[:, s * 8:(s + 1) * 8],
                                         start=False, stop=False)
                        for j in range(NJ):
                            nc.tensor.matmul(
                                acc[:, j * 8:(j + 1) * 8],
                                xq[:, s * DIM + j * 128: s * DIM + (j + 1) * 128],
                                at[:, s * 8:(s + 1) * 8],
                                start=False,
                                stop=(j == NJ - 1),
                            )

            def batch_tail(b):
                acc = accs[b]
                h0 = b * HEADS
                nc.vector.tensor_copy(sums_sb[0:1, h0:h0 + 8], acc[0:1, 32:40])
                ps_bc = ps_sm.tile([128, 32], F32, tag="sm", name="ps_bc")
                nc.tensor.matmul(ps_bc[:, 0:8], obc_sb,
                                 sums_sb[0:1, h0:h0 + 8])
                nc.vector.reciprocal(sbc_sb[:, h0:h0 + 8], ps_bc[:, 0:8])
                for j in range(NJ):
                    nc.vector.tensor_mul(
                        xbarT[:, b * 32 + j * 8: b * 32 + (j + 1) * 8],
                        acc[:, j * 8:(j + 1) * 8],
                        sbc_sb[:, h0:h0 + 8],
                    )
                for ji in range(NJ):
                    pv = ps_sm.tile([128, 32], F32, tag="sm", name="pv")
                    for jd in range(NJ):
                        nc.tensor.matmul(
                            pv[:, 0:8],
                            c16[:, C_WVT + jd * DIM + ji * 128: C_WVT + jd * DIM + (ji + 1) * 128],
                            xbarT[:, b * 32 + jd * 8: b * 32 + (jd + 1) * 8],
                            start=(jd == 0),
                            stop=(jd == NJ - 1),
                        )
                    # head-block-diagonal extract: chunk ji covers heads 2ji
                    # (rows 0-63) and 2ji+1 (rows 64-127)
                    c = b * NJ + ji
                    nc.vector.tensor_copy(v_all[0:64, c:c + 1],
                                          pv[0:64, 2 * ji:2 * ji + 1])
                    nc.vector.tensor_copy(v_all[64:128, c:c + 1],
                                          pv[64:128, 2 * ji + 1:2 * ji + 2])
                for jd in range(NJ):
                    po = ps_sm.tile([128, 32], F32, tag="sm", name="po")
                    for ji in range(NJ):
                        nc.tensor.matmul(
                            po[:, 0:1],
                            c16[:, C_WOT + ji * DIM + jd * 128: C_WOT + ji * DIM + (jd + 1) * 128],
                            v_all[:, b * NJ + ji: b * NJ + ji + 1],
                            start=(ji == 0),
                            stop=(ji == NJ - 1),
                        )
                    nc.vector.tensor_tensor(
                        o_sb[:, b * NJ + jd: b * NJ + jd + 1],
                        po[:, 0:1], c32[:, C_BO + jd:C_BO + jd + 1],
                        mybir.AluOpType.add,
                    )

            # 4-deep pipeline: scores trail the copies by two iterations so
            # the PE never stalls on the same-iteration copy round trip
            nitems = BPC * NQ
            for i in range(nitems + 4):
                if i < nitems:
                    stage_a(i)
                if 1 <= i < nitems + 1:
                    stage_b(i - 1)
                if 3 <= i < nitems + 3:
                    stage_c(i - 3)
                if 4 <= i < nitems + 4:
                    k = i - 4
                    stage_d(k)
                    b, q = divmod(k, NQ)
                    if q == NQ - 1:
                        batch_tail(b)
                # batch-0 output DMA a few iterations after its tail so the
                # SP sequencer never parks on it
                if i == nitems - 2:
                    nc.sync.dma_start(out_d.ap()[:, 0:NJ], o_sb[:, 0:NJ])
            nc.sync.dma_start(out_d.ap()[:, NJ:2 * NJ], o_sb[:, NJ:2 * NJ])

    nc.compile()
    return nc


def kernel(**inputs):
    global LAST_RESULTS
    x = np.ascontiguousarray(np.asarray(inputs["x"], dtype=np.float32))
    Wq = np.asarray(inputs["Wq"], dtype=np.float32)
    Wk = np.asarray(inputs["Wk"], dtype=np.float32)
    Wv = np.asarray(inputs["Wv"], dtype=np.float32)
    Wo = np.asarray(inputs["Wo"], dtype=np.float32)
    bo = np.asarray(inputs["bo"], dtype=np.float32)
    pi = np.asarray(inputs["patch_indices"]).astype(np.int64)
    scale = np.asarray(inputs["scale"]).astype(np.int64)

    idx = pi[:, 0] * scale[1] + pi[:, 1]
    sel = x[np.arange(B), idx]                       # [B, DIM]
    q = (sel @ Wq.T).reshape(B, HEADS, DHEAD)        # [B, h, dh]
    # ck[b, d, h] = sum_i q[b,h,i] * Wk[h*64+i, d], scaled by 1/sqrt(dh)
    ck = np.einsum("bhi,hid->bdh", q, Wk.reshape(HEADS, DHEAD, DIM),
                   dtype=np.float32).astype(np.float32) * np.float32(DHEAD ** -0.5)

    wvt = Wv.T.reshape(NJ, 128, DIM).transpose(1, 0, 2).reshape(128, NJ * DIM)
    wot = Wo.T.reshape(NJ, 128, DIM).transpose(1, 0, 2).reshape(128, NJ * DIM)

    c32 = np.zeros((128, W32), dtype=np.float32)
    c32[:, C_BO:C_BO + NJ] = bo.reshape(NJ, 128).T
    c32[0, C_OBC:C_OBC + 128] = 1.0

    x16 = x.astype(np.float16)
    in_maps = []
    for c in range(NCORES):
        xsf = x16[c * BPC:(c + 1) * BPC]             # [2, 4096, 512] fp16
        xs_nat = xsf.reshape(BPC, NQ, NT, 128, DIM).transpose(0, 1, 3, 2, 4)
        xs = np.ascontiguousarray(xs_nat.reshape(BPC, NQ, 128, NT * DIM))
        c16 = np.zeros((128, W16), dtype=np.float16)
        c16[:, C_ID:C_ID + 128] = np.eye(128, dtype=np.float16)
        for bb in range(BPC):
            for j in range(NJ):
                c16[:, C_CK + (bb * NJ + j) * HEADS:C_CK + (bb * NJ + j + 1) * HEADS] = \
                    ck[c * BPC + bb, j * 128:(j + 1) * 128, :].astype(np.float16)
        c16[:, C_ONES] = 1.0
        c16[:, C_WVT:C_WVT + NJ * DIM] = wvt.astype(np.float16)
        c16[:, C_WOT:C_WOT + NJ * DIM] = wot.astype(np.float16)
        in_maps.append({"x": xs, "c16": c16, "c32": c32})

    nc = build_program()
    res = run_bass_kernel_spmd(nc, in_maps, list(range(NCORES)), trace=TRACE)
    LAST_RESULTS = res

    out = np.empty((B, 1, DIM), dtype=np.float32)
    for c in range(NCORES):
        oc = res.results[c]["out"]                   # [128, NJ*BPC]
        for bb in range(BPC):
            out[c * BPC + bb, 0, :] = oc[:, bb * NJ:(bb + 1) * NJ].T.reshape(DIM)
    return out


# revision 14
# speedup vs baseline: 1.4419x; 1.0282x over previous
"""CenterPixelCrossAttention Trainium2 kernel (v3).

Math: each batch item has a SINGLE query token (the center pixel), so the
attention collapses to rank-1 form:
    scores[b, t, h] = x[b, t, :] . ck[b, :, h]   with ck[b] = (Wk_h^T q_{b,h}) * sm_scale
    xbar[b, h, :]   = sum_t exp(scores[b,t,h]) * x[b, t, :]   (unnormalized)
    out[b]          = concat_h((Wv_h @ xbar_{b,h}) / S_{b,h}) @ Wo^T + bo
The full K/V projections are never materialized; x streams from HBM exactly
once in fp16 NATIVE layout only (8.4 MB/core, the model-bandwidth floor at
~360 GB/s).

Key structure:
  - every x-consuming matmul has out free size 8 (stationary loads are
    cheap; cost ~ output columns):
      scores^T[t, h]: lhsT = x^T chunk [128d, 128t], rhs = ck [128d, 8h]
      xbar^T [d, h]:  lhsT = x  chunk [128t, 128d], rhs = at [128t, 8h]
      sums  [1, h]:   lhsT = ones [128t, 1],        rhs = at [128t, 8h]
  - x^T chunks are produced on-chip: PE is_transpose into PSUM, DVE copies
    to SBUF; ACT only does the exp.  All weights/constants arrive in two
    blob DMAs so the sequencer issues the 16 x-quad streams back to back.
  - xbar/sums PSUM accumulators live across a whole batch; only the very
    first matmul touching the bank uses start=True and only the very last
    uses stop=True (PSUM pending-zero is per byte, so each column group
    zero-fills lazily on first touch and accumulates afterwards).
  - softmax is max-free (logits are O(1) by construction); 1/sum folds in
    at the per-batch tail via a ones-broadcast matmul + DVE multiply.

Distribution: data-parallel over batch, 2 batch items per NeuronCore, no
collectives.  3-stage software pipeline over 16 (batch, quad) items:
  A(k): DMA quad k            B(k): PE transposes + DVE copies
  C(k): scores + exp          D(k): xbar/sums accumulation (+ batch tail)
"""

import numpy as np
from contextlib import ExitStack

import concourse.bass as bass
import concourse.bacc as bacc
import concourse.tile as tile
from concourse import mybir
from concourse.bass_utils import run_bass_kernel_spmd

F32 = mybir.dt.float32
F16 = mybir.dt.float16

B, N, DIM, HEADS, DHEAD = 16, 4096, 512, 8, 64
NCORES = 8
BPC = B // NCORES          # batch items per core = 2
NQ = 8                     # quads (512-token groups) per batch item
QT = 512                   # tokens per quad
NT = 4                     # 128-token sub-tiles per quad
NJ = 4                     # 128-wide d-chunks
QW = QT * NT               # 2048 fp16 columns per quad (native layout only)

# f16 constant blob column offsets: ident | ck | ones | wvt | wot
C_ID = 0
C_CK = 128
C_ONES = C_CK + BPC * NJ * HEADS      # 192
C_WVT = C_ONES + 1                    # 193
C_WOT = C_WVT + NJ * DIM              # 2241
W16 = C_WOT + NJ * DIM                # 4289
# f32 constant blob: bo (4 cols) | onesbc row (128 cols, row 0 only)
C_BO = 0
C_OBC = NJ                            # 4
W32 = NJ + 128                        # 132

TRACE = False              # test.py flips this for profiling runs
LAST_RESULTS = None        # stash of BassKernelResults for test.py


def build_program(reps=1):
    DT = F16
    nc = bacc.Bacc("TRN2", target_bir_lowering=False, debug=False,
                   num_devices=NCORES)

    x_d = nc.dram_tensor("x", [BPC, NQ, 128, QW], DT, kind="ExternalInput")
    c16_d = nc.dram_tensor("c16", [128, W16], DT, kind="ExternalInput")
    c32_d = nc.dram_tensor("c32", [128, W32], F32, kind="ExternalInput")
    out_d = nc.dram_tensor("out", [128, NJ * BPC], F32, kind="ExternalOutput")

    with tile.TileContext(nc) as tc, ExitStack() as ctx:
        const = ctx.enter_context(tc.tile_pool(name="const", bufs=1))
        xq_pool = ctx.enter_context(tc.tile_pool(name="xq", bufs=BPC * NQ))
        xt_pool = ctx.enter_context(tc.tile_pool(name="xt", bufs=BPC * NQ))
        at_pool = ctx.enter_context(tc.tile_pool(name="at", bufs=4))
        ps_big = ctx.enter_context(tc.tile_pool(name="ps_big", bufs=4, space="PSUM"))
        ps_sm = ctx.enter_context(tc.tile_pool(name="ps_sm", bufs=2, space="PSUM"))
        ps_acc = ctx.enter_context(tc.tile_pool(name="ps_acc", bufs=2, space="PSUM"))

        c16 = const.tile([128, W16], DT)
        c32 = const.tile([128, W32], F32)
        ident = c16[:, C_ID:C_ID + 128]
        ones_sb = c16[:, C_ONES:C_ONES + 1]
        obc_sb = c32[0:1, C_OBC:C_OBC + 128]

        sums_sb = const.tile([1, BPC * HEADS], F32)
        sbc_sb = const.tile([128, BPC * HEADS], F32)
        xbarT = const.tile([128, BPC * NJ * HEADS], DT)
        v_all = const.tile([128, BPC * NJ], DT)
        o_sb = const.tile([128, BPC * NJ], F32)

        # two blob DMAs for every weight/constant: keeps the SP sequencer
        # free to issue the 16 x-quad streams back to back afterwards
        nc.sync.dma_start(c16[:], c16_d.ap()[:, :])
        nc.sync.dma_start(c32[:], c32_d.ap()[:, :])

        for _rep in range(reps):
            xqs = {}
            xts = {}
            ats = {}
            accs = {}

            def stage_a(k):
                b, q = divmod(k, NQ)
                xq = xq_pool.tile([128, QW], DT, tag="xq")
                xqs[k] = xq
                nc.sync.dma_start(xq[:], x_d.ap()[b, q])

            def stage_b(k):
                xq = xqs[k]
                xt = xt_pool.tile([128, QW], DT, tag="xt")
                xts[k] = xt
                for jh in range(2):          # two half-quads: j in {0,1}, {2,3}
                    pb = ps_big.tile([128, QW // 2], DT, tag="pb", name="pb")
                    for jj in range(2):
                        j = jh * 2 + jj
                        for s in range(NT):
                            nc.tensor.matmul(
                                pb[:, jj * QT + s * 128: jj * QT + (s + 1) * 128],
                                xq[:, s * DIM + j * 128: s * DIM + (j + 1) * 128],
                                ident,
                                is_transpose=True,
                            )
                    if jh == 0:
                        nc.vector.tensor_copy(xt[:, 0:QT * 2], pb[:])
                    else:
                        nc.scalar.copy(xt[:, QT * 2:QT * 4], pb[:])

            def stage_c(k):
                b, q = divmod(k, NQ)
                xt = xts[k]
                ps_s = ps_sm.tile([128, 32], F32, tag="sm", name="ps_s")
                for s in range(NT):
                    for j in range(NJ):
                        nc.tensor.matmul(
                            ps_s[:, s * 8:(s + 1) * 8],
                            xt[:, j * QT + s * 128: j * QT + (s + 1) * 128],
                            c16[:, C_CK + (b * NJ + j) * 8: C_CK + (b * NJ + j + 1) * 8],
                            start=(j == 0),
                            stop=(j == NJ - 1),
                        )
                at = at_pool.tile([128, 32], DT, tag="at", name="at")
                ats[k] = at
                nc.scalar.activation(at[:], ps_s[:],
                                     mybir.ActivationFunctionType.Exp)

            def stage_d(k):
                b, q = divmod(k, NQ)
                xq, at = xqs[k], ats[k]
                if q == 0:
                    accs[b] = ps_acc.tile([128, 64], F32, tag="acc",
                                          name=f"acc{b}")
                acc = accs[b]
                # one PSUM bank, five column groups (4x xbar_j + sums).
                # start=True only on the very first matmul touching the bank
                # (marks the whole zero region pending; each group then
                # lazily zero-fills its own bytes on first touch), stop=True
                # only on the very last.
                for s in range(NT):
                    last = (q == NQ - 1 and s == NT - 1)
                    if not last:
                        # sums after the j-loop in steady state
                        for j in range(NJ):
                            nc.tensor.matmul(
                                acc[:, j * 8:(j + 1) * 8],
                                xq[:, s * DIM + j * 128: s * DIM + (j + 1) * 128],
                                at[:, s * 8:(s + 1) * 8],
                                start=(q == 0 and s == 0 and j == 0),
                                stop=False,
                            )
                        nc.tensor.matmul(acc[0:1, 32:40], ones_sb,
                                         at[:, s * 8:(s + 1) * 8],
                                         start=False, stop=False)
                    else:
                        # final step: sums first, then close the group with a
                        # full-128-partition matmul so every partition's zero
                        # region is released before the tail reads it
                        nc.tensor.matmul(acc[0:1, 32:40], ones_sb,
                                         at[:, s * 8:(s + 1) * 8],
                                         start=False, stop=False)
                        for j in range(NJ):
                            nc.tensor.matmul(
                                acc[:, j * 8:(j + 1) * 8],
                                xq[:, s * DIM + j * 128: s * DIM + (j + 1) * 128],
                                at[:, s * 8:(s + 1) * 8],
                                start=False,
                                stop=(j == NJ - 1),
                            )

            def batch_tail_a(b):
                acc = accs[b]
                h0 = b * HEADS
                nc.vector.tensor_copy(sums_sb[0:1, h0:h0 + 8], acc[0:1, 32:40])
                ps_bc = ps_sm.tile([128, 32], F32, tag="sm", name="ps_bc")
                nc.tensor.matmul(ps_bc[:, 0:8], obc_sb,
                                 sums_sb[0:1, h0:h0 + 8])
                nc.vector.reciprocal(sbc_sb[:, h0:h0 + 8], ps_bc[:, 0:8])
                # single normalize: acc[:, j*8+h] * sinv[h] via stride-0
                # column repeat of the sinv row block
                sb8 = sbc_sb[:, h0:h0 + 8]
                rep = bass.AP(sb8.tensor, sb8.offset,
                              [list(sb8.ap[0]), [0, NJ], list(sb8.ap[1])])
                a32 = acc[:, 0:32]
                a3 = bass.AP(a32.tensor, a32.offset,
                             [list(a32.ap[0]), [8, NJ], [1, 8]])
                xo = xbarT[:, b * 32:(b + 1) * 32]
                x3 = bass.AP(xo.tensor, xo.offset,
                             [list(xo.ap[0]), [8, NJ], [1, 8]])
                nc.vector.tensor_tensor(x3, a3, rep, mybir.AluOpType.mult)

            def batch_tail_v(b, jis):
                for ji in jis:
                    pv = ps_sm.tile([128, 32], F32, tag="sm", name="pv")
                    for jd in range(NJ):
                        nc.tensor.matmul(
                            pv[:, 0:8],
                            c16[:, C_WVT + jd * DIM + ji * 128: C_WVT + jd * DIM + (ji + 1) * 128],
                            xbarT[:, b * 32 + jd * 8: b * 32 + (jd + 1) * 8],
                            start=(jd == 0),
                            stop=(jd == NJ - 1),
                        )
                    # head-block-diagonal extract: chunk ji covers heads 2ji
                    # (rows 0-63) and 2ji+1 (rows 64-127); split DVE/ACT
                    c = b * NJ + ji
                    nc.vector.tensor_copy(v_all[0:64, c:c + 1],
                                          pv[0:64, 2 * ji:2 * ji + 1])
                    nc.vector.tensor_copy(v_all[64:128, c:c + 1],
                                          pv[64:128, 2 * ji + 1:2 * ji + 2])

            def batch_tail_b(b):
                for jd in range(NJ):
                    po = ps_sm.tile([128, 32], F32, tag="sm", name="po")
                    for ji in range(NJ):
                        nc.tensor.matmul(
                            po[:, 0:1],
                            c16[:, C_WOT + ji * DIM + jd * 128: C_WOT + ji * DIM + (jd + 1) * 128],
                            v_all[:, b * NJ + ji: b * NJ + ji + 1],
                            start=(ji == 0),
                            stop=(ji == NJ - 1),
                        )
                    if jd % 2 == 0:
                        nc.vector.tensor_tensor(
                            o_sb[:, b * NJ + jd: b * NJ + jd + 1],
                            po[:, 0:1], c32[:, C_BO + jd:C_BO + jd + 1],
                            mybir.AluOpType.add,
                        )
                    else:
                        nc.scalar.activation(
                            o_sb[:, b * NJ + jd: b * NJ + jd + 1], po[:, 0:1],
                            mybir.ActivationFunctionType.Identity,
                            bias=c32[:, C_BO + jd:C_BO + jd + 1],
                        )

            # 4-deep pipeline: scores trail the copies by two iterations so
            # the PE never stalls on the same-iteration copy round trip
            nitems = BPC * NQ
            for i in range(nitems + 4):
                if i < nitems:
                    stage_a(i)
                if 1 <= i < nitems + 1:
                    stage_b(i - 1)
                if 3 <= i < nitems + 3:
                    stage_c(i - 3)
                if 4 <= i < nitems + 4:
                    k = i - 4
                    stage_d(k)
                    b, q = divmod(k, NQ)
                    if q == NQ - 1:
                        batch_tail_a(b)
                # Wv/Wo tail phases each deferred one iteration so their PE
                # matmuls (which wait on DVE-side chains) never head-of-line
                # block the next iteration's transposes
                if i == NQ + 4 + 1:
                    batch_tail_v(0, (0, 1))
                if i == NQ + 4 + 2:
                    batch_tail_v(0, (2, 3))
                if i == NQ + 4 + 3:
                    batch_tail_b(0)
                if i == nitems - 1:
                    # batch-0 output DMA after the last x-quad issue so the
                    # SP sequencer never delays the stream by parking on it
                    nc.sync.dma_start(out_d.ap()[:, 0:NJ], o_sb[:, 0:NJ])
            batch_tail_v(1, (0, 1, 2, 3))
            batch_tail_b(1)
            nc.sync.dma_start(out_d.ap()[:, NJ:2 * NJ], o_sb[:, NJ:2 * NJ])

    nc.compile()
    return nc


def kernel(**inputs):
    global LAST_RESULTS
    x = np.ascontiguousarray(np.asarray(inputs["x"], dtype=np.float32))
    Wq = np.asarray(inputs["Wq"], dtype=np.float32)
    Wk = np.asarray(inputs["Wk"], dtype=np.float32)
    Wv = np.asarray(inputs["Wv"], dtype=np.float32)
    Wo = np.asarray(inputs["Wo"], dtype=np.float32)
    bo = np.asarray(inputs["bo"], dtype=np.float32)
    pi = np.asarray(inputs["patch_indices"]).astype(np.int64)
    scale = np.asarray(inputs["scale"]).astype(np.int64)

    idx = pi[:, 0] * scale[1] + pi[:, 1]
    sel = x[np.arange(B), idx]                       # [B, DIM]
    q = (sel @ Wq.T).reshape(B, HEADS, DHEAD)        # [B, h, dh]
    # ck[b, d, h] = sum_i q[b,h,i] * Wk[h*64+i, d], scaled by 1/sqrt(dh)
    ck = np.einsum("bhi,hid->bdh", q, Wk.reshape(HEADS, DHEAD, DIM),
                   dtype=np.float32).astype(np.float32) * np.float32(DHEAD ** -0.5)

    wvt = Wv.T.reshape(NJ, 128, DIM).transpose(1, 0, 2).reshape(128, NJ * DIM)
    wot = Wo.T.reshape(NJ, 128, DIM).transpose(1, 0, 2).reshape(128, NJ * DIM)

    c32 = np.zeros((128, W32), dtype=np.float32)
    c32[:, C_BO:C_BO + NJ] = bo.reshape(NJ, 128).T
    c32[0, C_OBC:C_OBC + 128] = 1.0

    x16 = x.astype(np.float16)
    in_maps = []
    for c in range(NCORES):
        xsf = x16[c * BPC:(c + 1) * BPC]             # [2, 4096, 512] fp16
        xs_nat = xsf.reshape(BPC, NQ, NT, 128, DIM).transpose(0, 1, 3, 2, 4)
        xs = np.ascontiguousarray(xs_nat.reshape(BPC, NQ, 128, NT * DIM))
        c16 = np.zeros((128, W16), dtype=np.float16)
        c16[:, C_ID:C_ID + 128] = np.eye(128, dtype=np.float16)
        for bb in range(BPC):
            for j in range(NJ):
                c16[:, C_CK + (bb * NJ + j) * HEADS:C_CK + (bb * NJ + j + 1) * HEADS] = \
                    ck[c * BPC + bb, j * 128:(j + 1) * 128, :].astype(np.float16)
        c16[:, C_ONES] = 1.0
        c16[:, C_WVT:C_WVT + NJ * DIM] = wvt.astype(np.float16)
        c16[:, C_WOT:C_WOT + NJ * DIM] = wot.astype(np.float16)
        in_maps.append({"x": xs, "c16": c16, "c32": c32})

    nc = build_program()
    res = run_bass_kernel_spmd(nc, in_maps, list(range(NCORES)), trace=TRACE)
    LAST_RESULTS = res

    out = np.empty((B, 1, DIM), dtype=np.float32)
    for c in range(NCORES):
        oc = res.results[c]["out"]                   # [128, NJ*BPC]
        for bb in range(BPC):
            out[c * BPC + bb, 0, :] = oc[:, bb * NJ:(bb + 1) * NJ].T.reshape(DIM)
    return out
